# revision 11
# baseline (speedup 1.0000x reference)
"""Trainium2 Bass kernel for DifferentiableMVOLayer (batched simplex-constrained QP).

Per-sample FISTA solve of  min -mu'w + (lam/2) w'(U^T U)w  s.t. w in simplex.
Data-parallel over 8 NeuronCores (16 samples each).

Key design points:
  - U is shipped to the device as int8 (quarters the dominant cost: host->device
    transfer over the axon tunnel at ~70MB/s). Per-sample symmetric scale
    s_b = max|U_b|/127; the QP is solved with the unscaled integer Gram matrix
    Q' = q^T q and mu rescaled to mu/s_b^2 on the host, which has the same
    argmin (objective scaling). q values dequantize exactly to fp16 and the
    fp16 PE matmul of integers accumulated in fp32 PSUM is exact, so the only
    error is the quantization of U itself (~1.1e-2 rel end-to-end, inside the
    2e-2 gate; fp16 wire format gives ~2e-4 if more margin is ever needed).
  - Per-core chunks of U are quantized and device_put one at a time, so the
    (async, C++-thread) tunnel transfer of chunk c overlaps quantization of
    chunk c+1.
  - The PJRT executable (jit of shard_map over the bass_exec custom call) is
    built ONCE and cached at module level; repeat calls skip re-trace /
    re-lower / re-compile and go straight to H2D + execute + D2H.
  - Full-batch arrays are passed straight to the sharded jit (axis 0 is
    already the core axis) — no host-side split + re-concat copies.
  - Batched matvec Q@y via masked-stationary trick: stationary [128,8] holds one
    sample's y slice in column b (zeros elsewhere) so 32 matmuls + an identity
    matmul folding -mu/lam accumulate all 8 samples' results into one PSUM tile
    [8, 512] in natural layout.
  - Simplex projection via warm-started Michelot threshold iteration
    (exact after a few steps), all on the vector engine with fused
    scalar_tensor_tensor/accum ops.
  - FISTA momentum scalars are input-independent -> baked in as immediates.
  - Two 8-sample groups pipeline PE (matvec) against DVE (projection).
"""

import math
import numpy as np

N_ASSETS = 512
BATCH = 128
N_CORES = 8
B_CORE = BATCH // N_CORES          # 16 samples per core
GRP = 8                            # samples per pipeline group
N_GROUPS = B_CORE // GRP
LAMBDA = 10.0
FISTA_ITERS = 128
POWER_ITERS = 4
L_MARGIN = 1.08                    # L safety factor (fewer power iters)
MICH_COLD = 8                      # Michelot iters, first FISTA step
MICH_WARM = 2                      # Michelot iters, warm-started steps
NT = N_ASSETS // 128               # 4 j-tiles

_CACHE = {}
_RUN = {}


def _momentum_coeffs(n_iters):
    t = np.float32(1.0)
    cs = []
    for _ in range(n_iters):
        t_new = np.float32(0.5 * (1.0 + np.sqrt(np.float32(1.0 + 4.0 * t * t))))
        cs.append(float((t - np.float32(1.0)) / t_new))
        t = t_new
    return cs


def _build(n_fista, n_power):
    import concourse.bass as bass
    import concourse.mybir as mybir
    import concourse.tile as tile
    import concourse.bacc as bacc

    F32 = mybir.dt.float32
    F32R = mybir.dt.float32r
    F16 = mybir.dt.float16
    I8 = mybir.dt.int8
    OP = mybir.AluOpType

    nc = bacc.Bacc(trn_type="TRN2", target_bir_lowering=False)
    mu_d = nc.dram_tensor("mu", [B_CORE, N_ASSETS], F32, kind="ExternalInput")
    u_d = nc.dram_tensor("U", [B_CORE, N_ASSETS, N_ASSETS], I8,
                         kind="ExternalInput")
    w_d = nc.dram_tensor("W", [B_CORE, N_ASSETS], F32, kind="ExternalOutput")

    inv_sqrt_n = 1.0 / math.sqrt(N_ASSETS)
    cs = _momentum_coeffs(n_fista)

    with tile.TileContext(nc) as tc:
        with (
            tc.tile_pool(name="big", bufs=1) as big,
            tc.tile_pool(name="small", bufs=1) as small,
            tc.tile_pool(name="ps", bufs=1, space="PSUM") as ps,
        ):
            # ---------------- static tiles ----------------
            qall = big.tile([128, B_CORE, NT, N_ASSETS], F32R, name="qall")
            mu_sb = small.tile([B_CORE, N_ASSETS], F32, name="mu_sb")
            negmulam = small.tile([B_CORE, N_ASSETS], F32R, name="negmulam")
            zeros8 = small.tile([GRP, N_ASSETS], F32, name="zeros8")
            i16tmp = small.tile([16, 16], F32, name="i16tmp")
            i16f = small.tile([16, 16], F32, name="i16f")
            i16r = small.tile([16, 16], F32R, name="i16r")
            vinit = small.tile([128, NT, GRP], F32, name="vinit")

            nc.sync.dma_start(mu_sb[:], mu_d[:])
            nc.vector.memset(zeros8[:], 0.0)
            nc.gpsimd.iota(i16tmp[:], pattern=[[1, 16]], base=0,
                           channel_multiplier=-1,
                           allow_small_or_imprecise_dtypes=True)
            nc.vector.tensor_scalar(i16f[:], i16tmp[:], 0.0, None, OP.is_equal)
            nc.vector.tensor_copy(i16r[:], i16f[:])
            nc.vector.memset(vinit[:], inv_sqrt_n)
            nc.vector.tensor_scalar(negmulam[:], mu_sb[:], -1.0 / LAMBDA, None,
                                    OP.mult)
            i8 = i16f[0:GRP, 0:GRP]

            # per-group state
            ymask, yv, wv, wprev, vv, trash, yT = [], [], [], [], [], [], []
            th, rr, cc, rc, dth, nega, pv = [], [], [], [], [], [], []
            for g in range(N_GROUPS):
                ymask.append(big.tile([128, NT, GRP, GRP], F32R, name=f"ymask{g}"))
                yv.append(small.tile([GRP, N_ASSETS], F32, name=f"y{g}"))
                wv.append(small.tile([GRP, N_ASSETS], F32, name=f"w{g}"))
                wprev.append(small.tile([GRP, N_ASSETS], F32, name=f"wprev{g}"))
                vv.append(small.tile([GRP, N_ASSETS], F32, name=f"v{g}"))
                trash.append(small.tile([GRP, N_ASSETS], F32, name=f"trash{g}"))
                th.append(small.tile([GRP, 1], F32, name=f"th{g}"))
                rr.append(small.tile([GRP, 1], F32, name=f"r{g}"))
                cc.append(small.tile([GRP, 1], F32, name=f"c{g}"))
                rc.append(small.tile([GRP, 1], F32, name=f"rc{g}"))
                dth.append(small.tile([GRP, 1], F32, name=f"dth{g}"))
                nega.append(small.tile([GRP, 1], F32, name=f"nega{g}"))
                pv.append(ps.tile([GRP, N_ASSETS], F32, name=f"pv{g}"))
                yT.append(ps.tile([128, NT, GRP], F32, name=f"yT{g}"))

            def ym_diag(g):
                return ymask[g][:].rearrange("p t a b -> p t (a b)")[:, :, 0:GRP * GRP:GRP + 1]

            # ---------------- phase A: Q = U^T U ----------------
            with (
                tc.tile_pool(name="stage", bufs=2) as stage_pool,
                tc.tile_pool(name="qps", bufs=4, space="PSUM") as qps_pool,
            ):
                for s in range(B_CORE):
                    u8 = stage_pool.tile([128, NT, N_ASSETS], I8,
                                         name="u8", tag="u8")
                    ustage = stage_pool.tile([128, NT, N_ASSETS], F16,
                                             name="ustage", tag="ustage")
                    nc.sync.dma_start(
                        u8[:], u_d[s].rearrange("(t p) j -> p t j", p=128))
                    nc.vector.tensor_copy(ustage[:], u8[:])
                    for jm in range(NT):
                        qp = qps_pool.tile([128, N_ASSETS], F32, name="qp", tag="qp")
                        for it in range(NT):
                            nc.tensor.matmul(
                                qp[:], ustage[:, it, jm * 128:(jm + 1) * 128],
                                ustage[:, it, :],
                                start=(it == 0), stop=(it == NT - 1))
                        nc.vector.tensor_copy(qall[:, s, jm, :], qp[:])

            # ---------------- matvec helper ----------------
            def matvec(g, with_mu):
                for jt in range(NT):
                    for b in range(GRP):
                        s = g * GRP + b
                        last = (jt == NT - 1 and b == GRP - 1 and not with_mu)
                        nc.tensor.matmul(
                            pv[g][:], ymask[g][:, jt, :, b], qall[:, s, jt, :],
                            start=(jt == 0 and b == 0), stop=last)
                if with_mu:
                    nc.tensor.matmul(
                        pv[g][:], i16r[:, g * GRP:(g + 1) * GRP], negmulam[:],
                        start=False, stop=True)

            def retranspose(g, src):
                # src [GRP, 512] fp32 -> ymask diag (fp32r)
                for jt in range(NT):
                    nc.tensor.transpose(
                        yT[g][:, jt, :], src[:, jt * 128:(jt + 1) * 128], i8)
                nc.vector.tensor_copy(ym_diag(g), yT[g][:])

            # ---------------- phase B: power iteration ----------------
            qv = [small.tile([GRP, N_ASSETS], F32, name=f"qv{g}")
                  for g in range(N_GROUPS)]
            ss = [small.tile([GRP, 1], F32, name=f"ss{g}") for g in range(N_GROUPS)]
            sqs = [small.tile([GRP, 1], F32, name=f"sq{g}") for g in range(N_GROUPS)]

            for g in range(N_GROUPS):
                nc.vector.memset(ymask[g][:].bitcast(F32), 0.0)
                nc.vector.tensor_copy(ym_diag(g), vinit[:])

            for it in range(n_power):
                for g in range(N_GROUPS):
                    matvec(g, with_mu=False)
                for g in range(N_GROUPS):
                    nc.vector.tensor_copy(qv[g][:], pv[g][:])
                    nc.vector.scalar_tensor_tensor(
                        trash[g][:], qv[g][:], 0.0, qv[g][:], OP.add, OP.mult,
                        accum_out=ss[g][:])
                    nc.scalar.sqrt(sqs[g][:], ss[g][:])
                    nc.vector.tensor_scalar(sqs[g][:], sqs[g][:], 1e-12, None,
                                            OP.add)
                    nc.vector.reciprocal(rc[g][:], sqs[g][:])
                    nc.vector.tensor_scalar(qv[g][:], qv[g][:], rc[g][:], None,
                                            OP.mult)
                    retranspose(g, qv[g][:])

            # one more matvec, then Rayleigh quotient L = lam * (v'Qv)/(v'v) + eps
            num = [small.tile([GRP, 1], F32, name=f"num{g}") for g in range(N_GROUPS)]
            den = [small.tile([GRP, 1], F32, name=f"den{g}") for g in range(N_GROUPS)]
            for g in range(N_GROUPS):
                matvec(g, with_mu=False)
            for g in range(N_GROUPS):
                nc.vector.scalar_tensor_tensor(
                    trash[g][:], qv[g][:], 0.0, pv[g][:], OP.add, OP.mult,
                    accum_out=num[g][:])
                nc.vector.scalar_tensor_tensor(
                    trash[g][:], qv[g][:], 0.0, qv[g][:], OP.add, OP.mult,
                    accum_out=den[g][:])
                nc.vector.reciprocal(den[g][:], den[g][:])
                # lammax = num/den ; L = lam*lammax + 1e-6 ; nega = -lam/L
                nc.vector.tensor_scalar(num[g][:], num[g][:], den[g][:], None,
                                        OP.mult)
                nc.vector.tensor_scalar(num[g][:], num[g][:], LAMBDA * L_MARGIN,
                                        None, OP.mult)
                nc.vector.tensor_scalar(num[g][:], num[g][:], 1e-6, None,
                                        OP.add)
                nc.vector.reciprocal(num[g][:], num[g][:])
                nc.vector.tensor_scalar(nega[g][:], num[g][:], -LAMBDA, None,
                                        OP.mult)

            # ---------------- phase C: FISTA ----------------
            for g in range(N_GROUPS):
                nc.vector.memset(yv[g][:], 1.0 / N_ASSETS)
                nc.vector.memset(wprev[g][:], 1.0 / N_ASSETS)
                retranspose(g, yv[g][:])

            wcur, wold = wv, wprev
            for k in range(n_fista):
                ck = cs[k]
                for g in range(N_GROUPS):
                    matvec(g, with_mu=True)
                for g in range(N_GROUPS):
                    # v = y - a*P   (P = Qy - mu/lam, in PSUM)
                    if k == 0:
                        nc.vector.scalar_tensor_tensor(
                            vv[g][:], pv[g][:], nega[g][:], yv[g][:],
                            OP.mult, OP.add, accum_out=rr[g][:])
                        # cold start: th = (sum(v) - 1)/n
                        nc.vector.tensor_scalar(
                            th[g][:], rr[g][:], -1.0, None, OP.add)
                        nc.vector.tensor_scalar(
                            th[g][:], th[g][:], 1.0 / N_ASSETS, None, OP.mult)
                        n_mich = MICH_COLD
                    else:
                        nc.vector.scalar_tensor_tensor(
                            vv[g][:], pv[g][:], nega[g][:], yv[g][:],
                            OP.mult, OP.add)
                        n_mich = MICH_WARM
                    for _ in range(n_mich):
                        nc.vector.scalar_tensor_tensor(
                            trash[g][:], vv[g][:], th[g][:], zeros8[:],
                            OP.subtract, OP.max, accum_out=rr[g][:])
                        nc.vector.tensor_scalar(
                            trash[g][:], vv[g][:], th[g][:], 0.0,
                            OP.is_gt, OP.add, accum_out=cc[g][:])
                        nc.vector.reciprocal(rc[g][:], cc[g][:])
                        nc.vector.tensor_scalar(
                            dth[g][:], rr[g][:], -1.0, rc[g][:], OP.add, OP.mult)
                        nc.vector.tensor_tensor(
                            th[g][:], th[g][:], dth[g][:], OP.add)
                    # w = relu(v - th)
                    nc.vector.scalar_tensor_tensor(
                        wcur[g][:], vv[g][:], th[g][:], zeros8[:],
                        OP.subtract, OP.max)
                    if k < n_fista - 1:
                        # y = w + ck*(w - wold);  d stored in trash
                        nc.vector.tensor_tensor(
                            trash[g][:], wcur[g][:], wold[g][:], OP.subtract)
                        nc.vector.scalar_tensor_tensor(
                            yv[g][:], trash[g][:], ck, wcur[g][:],
                            OP.mult, OP.add)
                        retranspose(g, yv[g][:])
                wcur, wold = wold, wcur

            # ---------------- output: w / (sum(w) + 1e-12) ----------------
            wfin = wold  # last written group tiles
            for g in range(N_GROUPS):
                wout = small.tile([GRP, N_ASSETS], F32, name=f"wout{g}")
                nc.vector.tensor_scalar(
                    trash[g][:], wfin[g][:], 0.0, 0.0, OP.add, OP.add,
                    accum_out=rr[g][:])
                nc.vector.tensor_scalar(rr[g][:], rr[g][:], 1e-12, None, OP.add)
                nc.vector.reciprocal(rc[g][:], rr[g][:])
                nc.vector.tensor_scalar(
                    wout[:], wfin[g][:], rc[g][:], None, OP.mult)
                nc.sync.dma_start(w_d[g * GRP:(g + 1) * GRP, :], wout[:])

    nc.compile()
    return nc


def get_nc(n_fista=FISTA_ITERS, n_power=POWER_ITERS):
    key = (n_fista, n_power)
    if key not in _CACHE:
        _CACHE[key] = _build(n_fista, n_power)
    return _CACHE[key]


def _get_runner(n_fista=FISTA_ITERS, n_power=POWER_ITERS):
    """Build (once) a cached PJRT executable for the SPMD kernel.

    Mirrors concourse.bass2jax.run_bass_via_pjrt's multi-core path, but the
    jitted shard_map is kept alive across calls so repeat invocations skip
    re-trace/re-lower/re-compile, and full-batch arrays are fed directly
    (axis 0 = core axis) instead of being split and re-concatenated.
    """
    key = (n_fista, n_power)
    if key in _RUN:
        return _RUN[key]

    import jax
    from jax.experimental.shard_map import shard_map
    from jax.sharding import Mesh, PartitionSpec
    import concourse.mybir as mybir
    from concourse import bass2jax

    nc = get_nc(n_fista, n_power)
    bass2jax.install_neuronx_cc_hook()

    assert nc.dbg_addr is None
    partition_name = (nc.partition_id_tensor.name
                      if nc.partition_id_tensor else None)

    in_names, out_names, out_avals, zero_outs = [], [], [], []
    for alloc in nc.m.functions[0].allocations:
        if not isinstance(alloc, mybir.MemoryLocationSet):
            continue
        name = alloc.memorylocations[0].name
        if alloc.kind == "ExternalInput":
            if name != partition_name:
                in_names.append(name)
        elif alloc.kind == "ExternalOutput":
            shape = tuple(alloc.tensor_shape)
            dtype = mybir.dt.np(alloc.dtype)
            out_names.append(name)
            out_avals.append(jax.core.ShapedArray(shape, dtype))
            # full-batch donated zero output buffers (axis 0 = core axis)
            zero_outs.append(np.zeros((N_CORES * shape[0],) + shape[1:], dtype))
    n_params = len(in_names)
    n_outs = len(out_names)
    all_in_names = list(in_names) + list(out_names)
    if partition_name is not None:
        all_in_names.append(partition_name)

    def _body(*args):
        operands = list(args)
        if partition_name is not None:
            operands.append(bass2jax.partition_id_tensor())
        outs = bass2jax._bass_exec_p.bind(
            *operands,
            out_avals=tuple(out_avals),
            in_names=tuple(all_in_names),
            out_names=tuple(out_names),
            lowering_input_output_aliases=(),
            sim_require_finite=True,
            sim_require_nnan=True,
            nc=nc,
        )
        return tuple(outs)

    devices = jax.devices()[:N_CORES]
    assert len(devices) == N_CORES
    mesh = Mesh(np.asarray(devices), ("core",))
    donate = tuple(range(n_params, n_params + n_outs))
    fn = jax.jit(
        shard_map(_body, mesh=mesh,
                  in_specs=(PartitionSpec("core"),) * (n_params + n_outs),
                  out_specs=(PartitionSpec("core"),) * n_outs,
                  check_rep=False),
        donate_argnums=donate,
        keep_unused=True,
    )
    _RUN[key] = (fn, in_names, zero_outs)
    return _RUN[key]


def kernel(mu: np.ndarray, U: np.ndarray) -> np.ndarray:
    import jax
    from jax.sharding import Mesh, PartitionSpec, NamedSharding

    fn, in_names, zero_outs = _get_runner()

    U = np.asarray(U)
    mu = np.asarray(mu)
    # reused buffers: allocation + page-faulting dominates the arithmetic on
    # this 1-cpu host
    if "f32buf" not in _RUN:
        _RUN["f32buf"] = np.empty((B_CORE, N_ASSETS, N_ASSETS), np.float32)
        _RUN["i8buf"] = np.empty((BATCH, N_ASSETS, N_ASSETS), np.int8)
    f32buf, i8buf = _RUN["f32buf"], _RUN["i8buf"]

    devices = jax.devices()[:N_CORES]
    mesh = Mesh(np.asarray(devices), ("core",))
    sh = NamedSharding(mesh, PartitionSpec("core"))

    # Per-sample symmetric quantization scale; q = rint(U/s), mu' = mu/s^2.
    # Quantize per-core chunks and kick off each async device_put immediately,
    # so the tunnel streams chunk c while chunk c+1 quantizes on the cpu (the
    # scale reductions are also per-chunk so the first put issues asap).
    parts = []
    invs_parts = []
    for c in range(N_CORES):
        sl = slice(c * B_CORE, (c + 1) * B_CORE)
        uc = U[sl]
        smax = np.maximum(uc.max(axis=(1, 2)), -uc.min(axis=(1, 2)))
        smax = np.where(smax > 0, smax, 1.0).astype(np.float32)
        invs = np.float32(127.0) / smax
        invs_parts.append(invs)
        np.multiply(uc, invs[:, None, None], out=f32buf)
        np.rint(f32buf, out=f32buf)
        np.copyto(i8buf[sl], f32buf, casting="unsafe")
        parts.append(jax.device_put(i8buf[sl], devices[c]))
    u_arr = jax.make_array_from_single_device_arrays(
        (BATCH, N_ASSETS, N_ASSETS), sh, parts)

    invs = np.concatenate(invs_parts)
    mu32 = mu.astype(np.float32) * (invs * invs)[:, None]

    feed = {"mu": mu32, "U": u_arr}
    args = [feed[n] for n in in_names] + [np.zeros_like(z) for z in zero_outs]
    outs = fn(*args)
    return np.asarray(outs[0])
